# revision 13
# baseline (speedup 1.0000x reference)
"""HardTripletLoss (non-hardest branch) on 8 TRN2 NeuronCores.

Math:  loss = mean_{i!=j} relu(d_pos[i] - pdist[i,j] + margin)
  pdist[i,j] = ||x_i||^2 + ||y_j||^2 - 2 x_i.y_j ,  d_pos = diag(pdist)
  =>  per-term: relu(G[i,j] + a[i] - b[j]) with G = 2 x y^T,
      a[i] = margin + b[i] - G[i,i],  b[j] = ||y_j||^2  (xx cancels).
Diagonal (i==j) evaluates to ~relu(margin) = margin; the full unmasked sum is
computed and N*margin subtracted on the host.

Sharding: x rows split across 8 cores, y replicated.  Inputs arrive
pre-transposed/scaled from the host (bf16): xT2 = (2 x_shard)^T [128,1024],
ylT = y_shard^T [128,1024] (bit-identical to the matching yT slice),
yT = y^T [128,8192].  ~2.5 MB HBM per core, all HWDGE.

Per core, per 1024-col group n (8 groups), per m-tile (8):
  PSUM[128,1024] = xT2_m^T @ yT_n  chained with  -ones128^T @ sq_n
  (sq_n = square(yT_n) bf16, computed one col ahead on ACT/DVE alternating)
  so PSUM = G - b.  Epilogue alternates engines per m:
    DVE: sum_j max(PSUM + a_m, 0)   (STT vs a zeros tile, accum)
    ACT: sum_j relu(PSUM + a_m)     (activation w/ bias, accum)
a-path: sqc = square(ylT); prod = xT2*ylT elementwise; bbc/z2bb via
ones-matmuls; a16[1,1024] = (bbc + margin) - z2bb on partition 0;
PE-transpose (lhsT=a16 chunk, rhs=[1,1] ones) -> acol [128,8] f32.
The fold -ones@sq is the exact negation of +ones@sq (same PE summation
order), keeping a and the epilogue b consistent for the diagonal.
Host: loss = (sum(res) - N*margin) / N^2 in f64.
"""

import sys

if "/opt/trn_rl_repo" not in sys.path:
    sys.path.insert(0, "/opt/trn_rl_repo")

import numpy as np

N, D = 8192, 128
NCORES = 8
SH = N // NCORES          # 1024 x-rows per core
MT = SH // 128            # 8 m-tiles
NG = N // 1024            # 8 col groups of 1024
MARGIN = 0.2
# m-tile -> engine: even m = DVE (max vs 0), odd m = ACT (relu w/ bias).
# ACT tile first in each col (ACT also owns half the squares).
DVE_MS = tuple(m for m in range(MT) if m % 2 == 0)
ACT_MS = [m for m in range(MT) if m not in DVE_MS]
M_ORDER = [1, 0, 3, 2, 5, 4, 7, 6]

_cache = {}


def _build():
    import concourse.mybir as mybir
    from concourse import bacc
    from concourse.tile import TileContext
    from concourse.bass import ts

    f32 = mybir.dt.float32
    bf16 = mybir.dt.bfloat16
    Alu = mybir.AluOpType
    Act = mybir.ActivationFunctionType

    nc = bacc.Bacc()
    xT_in = nc.declare_dram_parameter("xT2", [128, SH], bf16, isOutput=False)
    ylT_in = nc.declare_dram_parameter("ylT", [128, SH], bf16, isOutput=False)
    yT_in = nc.declare_dram_parameter("yT", [128, N], bf16, isOutput=False)
    out_res = nc.declare_dram_parameter("res", [128, MT * NG], f32, isOutput=True)
    NDVE = len(DVE_MS)

    def sq_engine(n):
        return "act" if n % 2 == 0 else "dve"

    with TileContext(nc) as tc:
        with (
            tc.tile_pool(name="big", bufs=1) as big,
            tc.tile_pool(name="work", bufs=3) as work,
            tc.tile_pool(name="ps", bufs=1, space="PSUM") as ps,
        ):
            yTs = [big.tile([128, 1024], bf16, name=f"yT{n}") for n in range(NG)]
            xT = big.tile([128, SH], bf16)
            ylT = big.tile([128, SH], bf16)
            sqs = [big.tile([128, 1024], bf16, name=f"sq{n}") for n in range(NG)]
            sqc = big.tile([128, SH], bf16)
            ones128 = big.tile([128, 128], bf16)
            negones = big.tile([128, 128], bf16)
            ones1 = big.tile([1, 1], bf16)
            zeros = big.tile([128, 1024], f32)
            prod = big.tile([128, SH], bf16)
            a16 = big.tile([1, SH], bf16)
            z2r = big.tile([1, SH], f32)
            acol = big.tile([128, MT], f32)
            res_d = big.tile([128, NDVE * NG], f32)
            res_a = big.tile([128, (MT - NDVE) * NG], f32)

            nc.gpsimd.memset(ones128[:], 1.0)
            nc.gpsimd.memset(negones[:], -1.0)
            nc.gpsimd.memset(ones1[:], 1.0)
            nc.gpsimd.memset(zeros[:], 0.0)

            nc.sync.dma_start(xT[:], xT_in[:])
            nc.sync.dma_start(ylT[:], ylT_in[:])
            for n in range(NG):
                nc.sync.dma_start(yTs[n][:], yT_in[:, ts(n, 1024)])

            def do_sq(n):
                if sq_engine(n) == "act":
                    nc.scalar.activation(sqs[n][:], yTs[n][:], Act.Square)
                else:
                    nc.vector.scalar_tensor_tensor(
                        out=sqs[n][:], in0=yTs[n][:],
                        scalar=1.0, in1=yTs[n][:],
                        op0=Alu.mult, op1=Alu.mult,
                    )

            # ---- preamble: first squares + a-path ----
            do_sq(0)                     # ACT
            nc.vector.scalar_tensor_tensor(     # prod on DVE
                out=prod[:], in0=xT[:], scalar=1.0, in1=ylT[:],
                op0=Alu.mult, op1=Alu.mult,
            )
            nc.scalar.activation(sqc[:], ylT[:], Act.Square)
            do_sq(1)                     # DVE

            z2bb = ps.tile([128, 1024], f32, tag="bb")
            for h in range(2):
                nc.tensor.matmul(
                    z2bb[:, ts(h, 512)], lhsT=ones128[:],
                    rhs=prod[:, ts(h, 512)], start=True, stop=True,
                )
            bbc = ps.tile([128, 1024], f32, tag="g", bufs=3)
            for h in range(2):
                nc.tensor.matmul(
                    bbc[:, ts(h, 512)], lhsT=ones128[:],
                    rhs=sqc[:, ts(h, 512)], start=True, stop=True,
                )
            nc.scalar.activation(z2r[0:1, :], z2bb[0:1, :], Act.Copy)
            nc.vector.scalar_tensor_tensor(
                out=a16[0:1, :], in0=bbc[0:1, :], scalar=MARGIN,
                in1=z2r[0:1, :], op0=Alu.add, op1=Alu.subtract,
            )
            # transpose a16 -> acol via PE (stage in bb-shaped PSUM tile)
            tpt = ps.tile([128, 1024], f32, tag="bb")
            for m in range(MT):
                nc.tensor.matmul(
                    tpt[:, m : m + 1], lhsT=a16[0:1, ts(m, 128)],
                    rhs=ones1[:], start=True, stop=True,
                )
            for m in range(MT):
                nc.scalar.activation(acol[:, m : m + 1], tpt[:, m : m + 1], Act.Copy)

            # ---- main loop: fold -b into every PSUM tile ----
            for n in range(NG):
                for k, m in enumerate(M_ORDER):
                    is_dve = m in DVE_MS
                    pt = ps.tile([128, 1024], f32, tag="g", bufs=3)
                    for h in range(2):
                        nc.tensor.matmul(
                            pt[:, ts(h, 512)],
                            lhsT=xT[:, ts(m, 128)],
                            rhs=yTs[n][:, ts(h, 512)],
                            start=True, stop=False,
                        )
                        nc.tensor.matmul(
                            pt[:, ts(h, 512)],
                            lhsT=negones[:],
                            rhs=sqs[n][:, ts(h, 512)],
                            start=False, stop=True,
                        )
                    if is_dve:
                        di = n * NDVE + DVE_MS.index(m)
                        scr = work.tile([128, 1024], f32, tag="ep_dve")
                        nc.vector.scalar_tensor_tensor(
                            out=scr[:], in0=pt[:], scalar=acol[:, m : m + 1],
                            in1=zeros[:], op0=Alu.add, op1=Alu.max,
                            accum_out=res_d[:, di : di + 1],
                        )
                    else:
                        ai = n * (MT - NDVE) + ACT_MS.index(m)
                        scr = work.tile([128, 1024], f32, tag="ep_act")
                        nc.scalar.activation(
                            scr[:], pt[:], Act.Relu,
                            bias=acol[:, m : m + 1],
                            accum_out=res_a[:, ai : ai + 1],
                        )
                    # prefetch next-next col's squares mid-col
                    if k == 3 and n + 2 < NG:
                        do_sq(n + 2)

            nc.sync.dma_start(out_res[:, : NDVE * NG], res_d[:])
            nc.sync.dma_start(out_res[:, NDVE * NG :], res_a[:])

    return nc


def _make_in_maps(x: np.ndarray, y: np.ndarray) -> list:
    import ml_dtypes

    x = np.ascontiguousarray(x, dtype=np.float32)
    y = np.ascontiguousarray(y, dtype=np.float32)
    yb = y.astype(ml_dtypes.bfloat16)
    yT = np.ascontiguousarray(yb.T)
    in_maps = []
    for c in range(NCORES):
        sl = slice(c * SH, (c + 1) * SH)
        xT2 = np.ascontiguousarray((2.0 * x[sl]).astype(ml_dtypes.bfloat16).T)
        ylT = np.ascontiguousarray(yb[sl].T)
        in_maps.append({"xT2": xT2, "ylT": ylT, "yT": yT})
    return in_maps


def kernel(x: np.ndarray, y: np.ndarray) -> np.ndarray:
    from concourse.bass_utils import run_bass_kernel_spmd

    x = np.ascontiguousarray(x, dtype=np.float32)
    y = np.ascontiguousarray(y, dtype=np.float32)

    if "nc" not in _cache:
        nc = _build()
        if not nc.is_finalized():
            nc.finalize()
        _cache["nc"] = nc
    nc = _cache["nc"]

    out = run_bass_kernel_spmd(nc, _make_in_maps(x, y), list(range(NCORES)))
    results = out.results

    total = 0.0
    for c in range(NCORES):
        total += np.asarray(results[c]["res"], dtype=np.float64).sum()
    total -= float(N) * float(np.float32(MARGIN))
    return np.float32(total / (float(N) * float(N)))
